# revision 5
# baseline (speedup 1.0000x reference)
"""Trainium2 Bass kernel for nn_ClosedArap (ARAP rhs, GNN message passing).

rhs_i = sum_k w_ik * 0.5 * (R_i + R_j) @ (p_i - p_j),  j = nbr[i, k]

Design (8 NeuronCores, SPMD). Two facts about this environment drive it:
  * The axon link moves ~50-65 MB/s, so uploaded bytes dominate wall time
    (plus a fixed first-transfer penalty while the terminal drains prior
    session teardown - a tiny warm-up put starts that clock immediately).
  * The device's SWDGE indirect-DMA gather cannot batch multiple offsets
    per partition (multi-index offset APs scramble on HW), so an on-device
    neighbor gather costs ~1 us of Pool time per 128 edges (~8.5 ms/core)
    plus minutes of walrus compile for the thousands of DMA instructions.

So: vertices are sharded across cores; the random neighbor gather is
resolved on the host during staging (np.take over packed per-vertex
tables, ~0.3 s for all 8M edges); each core receives contiguous
partition-major streams and the device runs a ~130-instruction streaming
kernel: HWDGE loads, fp16 DVE edge math (int8 R_j payload dequantized on
the fly), a strided reduce over K=8, fp16 rhs store. Compiles in ~2 s and
executes in ~0.2 s including download.

Per-edge payload is 17 B: [p_j (3 fp16) | s_j (fp16)] packed stride-4 and
R_j as int8 scaled by the per-vertex s_j = max|R_j|/127. End-to-end
max-normalized error vs the f32 reference: ~1.8e-3 (tolerance 2e-2).

Wall-clock ordering inside kernel(): issue a warm-up transfer at t=0,
stage cores and fire async device_puts as each finishes, then bass-build
and AOT-compile (walrus) while the link drains, then execute on the
pre-uploaded shards. A conservative fallback reruns everything through
run_bass_kernel_spmd if the AOT fast path fails.
"""
import time

import numpy as np

import jax
from jax.sharding import Mesh, NamedSharding, PartitionSpec
from jax.experimental.shard_map import shard_map

from concourse import bass, bacc, bass2jax, mybir, tile

K = 8
NCORES = 8
D = 12           # fp16 local row: p(3) + R(9)
GRP = 123        # vertices per partition per group
NGRP = 8         # groups per core: 8*128*123 = 125952 >= 125000

LAST_EXEC_NS = None
LAST_RUN_WALL_S = None
LAST_STAGE_S = None
LAST_COMPILE_S = None
LAST_NEFF_S = None
LAST_UPLOAD_S = None
LAST_PATH = None

_IN_SHAPES = {
    "gps": ((128, NGRP * GRP * K * 4), np.float16),
    "gr8": ((128, NGRP * GRP * K * 9), np.int8),
    "wgt": ((128, NGRP * GRP * K), np.float16),
    "locpr": ((128, NGRP * GRP * D), np.float16),
}


def build_kernel(ngrp, grp, num_devices):
    nc = bacc.Bacc("TRN2", target_bir_lowering=False, debug=False,
                   num_devices=num_devices)
    f16 = mybir.dt.float16
    i8 = mybir.dt.int8
    ek = grp * K
    shp = 128 * grp * ngrp
    gps = nc.dram_tensor("gps", [128, ngrp * ek * 4], f16,
                         kind="ExternalInput").ap()
    gr8 = nc.dram_tensor("gr8", [128, ngrp * ek * 9], i8,
                         kind="ExternalInput").ap()
    wgt = nc.dram_tensor("wgt", [128, ngrp * ek], f16,
                         kind="ExternalInput").ap()
    locpr = nc.dram_tensor("locpr", [128, ngrp * grp * D], f16,
                           kind="ExternalInput").ap()
    rhs = nc.dram_tensor("rhs", [shp, 3], f16, kind="ExternalOutput").ap()

    with tile.TileContext(nc) as tc, tc.tile_pool(name="sbuf", bufs=2) as pool:
        for g in range(ngrp):
            ps_t = pool.tile([128, ek, 4], f16, tag="gps")
            r8_t = pool.tile([128, ek, 9], i8, tag="gr8")
            w_t = pool.tile([128, ek], f16, tag="wgt")
            pr_t = pool.tile([128, grp, D], f16, tag="locpr")
            rs_t = pool.tile([128, ek, 9], f16, tag="rsc")
            df_t = pool.tile([128, ek, 3], f16, tag="diff")
            s_t = pool.tile([128, ek, 9], f16, tag="ssum")
            u_t = pool.tile([128, ek, 3], f16, tag="utmp")
            t_t = pool.tile([128, ek, 3], f16, tag="tacc")
            m_t = pool.tile([128, ek, 3], f16, tag="mout")
            o2_t = pool.tile([128, grp, 3], f16, tag="out")

            nc.sync.dma_start(out=ps_t[:],
                              in_=gps[:, g * ek * 4:(g + 1) * ek * 4])
            nc.sync.dma_start(out=r8_t[:],
                              in_=gr8[:, g * ek * 9:(g + 1) * ek * 9])
            nc.sync.dma_start(out=w_t[:], in_=wgt[:, g * ek:(g + 1) * ek])
            nc.sync.dma_start(out=pr_t[:],
                              in_=locpr[:, g * grp * D:(g + 1) * grp * D])

            def pr_view(offset, width):
                # local row comp slice broadcast over k
                return bass.AP(
                    pr_t.tensor, pr_t[:].offset + offset,
                    [pr_t[:].ap[0], (D, grp), (0, K), (1, width)])

            # gathered p_j [128, ek, 3] (stride-4 rows of ps_t)
            gp = bass.AP(ps_t.tensor, ps_t[:].offset,
                         [ps_t[:].ap[0], (4, ek), (1, 3)])
            # per-edge scale s_j broadcast over the 9 R components
            sv = bass.AP(ps_t.tensor, ps_t[:].offset + 3,
                         [ps_t[:].ap[0], (4, ek), (0, 9)])

            # R_j = int8 * scale  (DVE converts the int8 operand on read)
            nc.vector.tensor_tensor(out=rs_t[:], in0=r8_t[:], in1=sv,
                                    op=mybir.AluOpType.mult)
            # diff = p_i - p_j ;  S = R_i + R_j
            nc.vector.tensor_tensor(out=df_t[:], in0=pr_view(0, 3),
                                    in1=gp, op=mybir.AluOpType.subtract)
            nc.vector.tensor_tensor(out=s_t[:], in0=pr_view(3, 9),
                                    in1=rs_t[:], op=mybir.AluOpType.add)

            def s_col(c):
                return bass.AP(s_t.tensor, s_t[:].offset + c,
                               [s_t[:].ap[0], (9, ek), (3, 3)])

            def d_col(c):
                return bass.AP(df_t.tensor, df_t[:].offset + c,
                               [df_t[:].ap[0], (3, ek), (0, 3)])

            # t = S @ diff (column-wise accumulation)
            nc.vector.tensor_tensor(out=t_t[:], in0=s_col(0), in1=d_col(0),
                                    op=mybir.AluOpType.mult)
            nc.vector.tensor_tensor(out=u_t[:], in0=s_col(1), in1=d_col(1),
                                    op=mybir.AluOpType.mult)
            nc.vector.tensor_tensor(out=t_t[:], in0=t_t[:], in1=u_t[:],
                                    op=mybir.AluOpType.add)
            nc.vector.tensor_tensor(out=u_t[:], in0=s_col(2), in1=d_col(2),
                                    op=mybir.AluOpType.mult)
            nc.vector.tensor_tensor(out=t_t[:], in0=t_t[:], in1=u_t[:],
                                    op=mybir.AluOpType.add)

            # m = t * w  (w pre-scaled by 0.5 on host; broadcast over 3 comps)
            wv = bass.AP(w_t.tensor, w_t[:].offset,
                         [w_t[:].ap[0], (1, ek), (0, 3)])
            nc.vector.tensor_tensor(out=m_t[:], in0=t_t[:], in1=wv,
                                    op=mybir.AluOpType.mult)

            # reduce over k (innermost view axis); fp16 accumulation of 8
            # terms costs ~1e-3 of max |rhs| against the 2e-2 tolerance
            mv = bass.AP(m_t.tensor, m_t[:].offset,
                         [m_t[:].ap[0], (3 * K, grp), (1, 3), (3, K)])
            with nc.allow_low_precision(reason="k=8 fp16 sum, tol 2e-2"):
                nc.vector.tensor_reduce(out=o2_t[:], in_=mv,
                                        axis=mybir.AxisListType.X,
                                        op=mybir.AluOpType.add)

            # contiguous store: partition p -> rhs rows [g*128*grp + p*grp ..)
            rhs_dst = bass.AP(rhs.tensor, g * 128 * grp * 3,
                              [(grp * 3, 128), (1, grp * 3)])
            nc.sync.dma_start(out=rhs_dst, in_=o2_t[:])
    nc.compile()
    return nc


def make_tables(xyz1, neighborList, weightMatrix, rotations, n):
    p16 = np.ascontiguousarray(xyz1[0]).astype(np.float16)
    r = np.ascontiguousarray(rotations).reshape(n, 9)
    r16 = r.astype(np.float16)
    table = np.concatenate([p16, r16], axis=1)            # [n, 12] fp16
    s = (np.abs(r).max(axis=1) / 127.0).astype(np.float16)
    table_ps = np.concatenate([p16, s[:, None]], axis=1)  # [n, 4] fp16
    table_r8 = np.clip(np.round(r / s.astype(np.float32)[:, None]),
                       -127, 127).astype(np.int8)         # [n, 9]
    nbr = np.ascontiguousarray(neighborList).reshape(n, K).astype(np.int32)
    w = (np.ascontiguousarray(weightMatrix).reshape(n, K)
         .astype(np.float16) * np.float16(0.5))
    return table, table_ps, table_r8, nbr, w


def stage_core(tables, i0, i1):
    table, table_ps, table_r8, nbr, w = tables
    shp = 128 * GRP * NGRP
    base = np.arange(shp)
    sh = i1 - i0
    vid = base % sh + i0                                  # padded ids (wrap)
    pad_mask = base >= sh

    def perm(a2d):
        # [shp, W] in vertex order -> [128, NGRP, GRP, W] partition-major
        W = a2d.shape[1]
        return np.ascontiguousarray(
            a2d.reshape(NGRP, 128, GRP, W).transpose(1, 0, 2, 3)
            .reshape(128, NGRP * GRP * W))

    nb_flat = perm(nbr[vid]).ravel()
    w_c = w[vid]
    w_c[pad_mask] = 0.0
    return {
        "gps": np.take(table_ps, nb_flat, axis=0).reshape(128, -1),
        "gr8": np.take(table_r8, nb_flat, axis=0).reshape(128, -1),
        "wgt": perm(w_c),
        "locpr": np.take(table, perm(vid[:, None]).ravel(),
                         axis=0).reshape(128, -1),
    }


def _exec_setup(nc):
    """Mirror run_bass_via_pjrt's multi-core path, AOT + presharded."""
    bass2jax.install_neuronx_cc_hook()
    partition_name = (nc.partition_id_tensor.name
                      if nc.partition_id_tensor else None)
    assert nc.dbg_addr is None
    in_names, out_names, out_avals = [], [], []
    for alloc in nc.m.functions[0].allocations:
        if not isinstance(alloc, mybir.MemoryLocationSet):
            continue
        name = alloc.memorylocations[0].name
        if alloc.kind == "ExternalInput":
            if name != partition_name:
                in_names.append(name)
        elif alloc.kind == "ExternalOutput":
            out_names.append(name)
            out_avals.append(jax.core.ShapedArray(
                tuple(alloc.tensor_shape), mybir.dt.np(alloc.dtype)))
    n_params = len(in_names)
    all_names = in_names + out_names
    if partition_name is not None:
        all_names = all_names + [partition_name]

    def _body(*args):
        operands = list(args)
        if partition_name is not None:
            operands.append(bass2jax.partition_id_tensor())
        outs = bass2jax._bass_exec_p.bind(
            *operands,
            out_avals=tuple(out_avals),
            in_names=tuple(all_names),
            out_names=tuple(out_names),
            lowering_input_output_aliases=(),
            sim_require_finite=True,
            sim_require_nnan=True,
            nc=nc,
        )
        return tuple(outs)

    devices = jax.devices()[:NCORES]
    mesh = Mesh(np.asarray(devices), ("core",))
    spec = PartitionSpec("core")
    n_out = len(out_names)
    sharded = jax.jit(
        shard_map(_body, mesh=mesh, in_specs=(spec,) * (n_params + n_out),
                  out_specs=(spec,) * n_out, check_rep=False),
        donate_argnums=tuple(range(n_params, n_params + n_out)),
        keep_unused=True,
    )
    return sharded, in_names, out_names, out_avals, mesh, spec, devices


_PROG = None       # (nc, setup) — bass program + jit wrapper, per process
_COMPILED = None   # AOT-compiled executable, per process


def _get_prog():
    global _PROG
    if _PROG is None:
        nc = build_kernel(NGRP, GRP, NCORES)
        _PROG = (nc, _exec_setup(nc))
    return _PROG


def _get_compiled(setup):
    """AOT-compile the sharded program once per process (cached)."""
    global _COMPILED
    if _COMPILED is not None:
        return _COMPILED
    sharded, in_names, out_names, out_avals, mesh, spec, devices = setup
    nds = NamedSharding(mesh, spec)
    global_avals = []
    for name in in_names:
        pc_shape, pc_dtype = _IN_SHAPES[name]
        global_avals.append(jax.ShapeDtypeStruct(
            (NCORES * pc_shape[0],) + pc_shape[1:], pc_dtype, sharding=nds))
    for av in out_avals:
        global_avals.append(jax.ShapeDtypeStruct(
            (NCORES * av.shape[0],) + av.shape[1:], av.dtype, sharding=nds))
    _COMPILED = sharded.lower(*global_avals).compile()
    return _COMPILED


def kernel(xyz1, xyz2, neighborList, numNeighbors, accnumNeighbors,
           weightMatrix, rotations, arapWeight, trace=False):
    global LAST_RUN_WALL_S, LAST_STAGE_S, LAST_COMPILE_S
    global LAST_NEFF_S, LAST_UPLOAD_S, LAST_PATH
    n = xyz1.shape[1]
    sh = n // NCORES
    shp = 128 * GRP * NGRP
    assert shp >= sh, (shp, sh)
    shard = [(c * sh, (c + 1) * sh) for c in range(NCORES)]

    # warm-up transfer: the first put of a process can stall for tens of
    # seconds while the terminal drains prior-session teardown; start that
    # clock before any CPU work.
    devices = jax.devices()[:NCORES]
    _warm = jax.device_put(np.zeros(1024, np.float32), devices[0])

    # stage each core and fire its uploads immediately (async): the link
    # drains while the next core stages and later while walrus compiles
    t0 = time.time()
    tables = make_tables(xyz1, neighborList, weightMatrix, rotations, n)
    core_maps = []
    shard_arrays = {name: [] for name in _IN_SHAPES}
    for c, (i0, i1) in enumerate(shard):
        cc = stage_core(tables, *shard[c])
        core_maps.append(cc)
        for name in shard_arrays:
            shard_arrays[name].append(jax.device_put(cc[name], devices[c]))
    t1 = time.time()
    LAST_STAGE_S = t1 - t0

    try:
        nc, setup = _get_prog()
        _, in_names, out_names, out_avals, mesh, spec, _devs = setup
        nds = NamedSharding(mesh, spec)
        t2 = time.time()
        LAST_COMPILE_S = t2 - t1
        out_zero_arrays = []
        for av in out_avals:
            z = np.zeros(av.shape, av.dtype)
            out_zero_arrays.append([jax.device_put(z, d) for d in devices])
        compiled = _get_compiled(setup)
        t3 = time.time()
        LAST_NEFF_S = t3 - t2

        for arrs in shard_arrays.values():
            for a in arrs:
                a.block_until_ready()
        for arrs in out_zero_arrays:
            for a in arrs:
                a.block_until_ready()
        t4 = time.time()
        LAST_UPLOAD_S = t4 - t3

        def _global(shards, pc_shape, dtype):
            gshape = (NCORES * pc_shape[0],) + tuple(pc_shape[1:])
            return jax.make_array_from_single_device_arrays(
                gshape, nds, shards)

        args = []
        for name in in_names:
            pc_shape, pc_dtype = _IN_SHAPES[name]
            args.append(_global(shard_arrays[name], pc_shape, pc_dtype))
        for i, av in enumerate(out_avals):
            args.append(_global(out_zero_arrays[i], av.shape, av.dtype))

        out_arrs = compiled(*args)
        rhs = np.asarray(out_arrs[0])
        LAST_RUN_WALL_S = time.time() - t4
        LAST_PATH = "aot"
    except Exception:
        # conservative fallback: stock SPMD runner (re-uploads everything)
        from concourse.bass_utils import run_bass_kernel_spmd
        nc = build_kernel(NGRP, GRP, NCORES)
        t3 = time.time()
        res = run_bass_kernel_spmd(nc, core_maps, list(range(NCORES)),
                                   trace=trace)
        LAST_RUN_WALL_S = time.time() - t3
        LAST_PATH = "fallback"
        rhs = np.concatenate([res.results[c]["rhs"] for c in range(NCORES)],
                             axis=0)

    rhs_g = np.asarray(rhs).reshape(NCORES, shp, 3)
    parts = [rhs_g[c, :sh] for c in range(NCORES)]
    return np.concatenate(parts, axis=0).astype(np.float32)


# revision 11
# speedup vs baseline: 1.2177x; 1.2177x over previous
"""Trainium2 Bass kernel for nn_ClosedArap (ARAP rhs, GNN message passing).

rhs_i = sum_k w_ik * 0.5 * (R_i + R_j) @ (p_i - p_j),  j = nbr[i, k]

Design (8 NeuronCores, SPMD). Two facts about this environment drive it:
  * The axon link moves ~50-65 MB/s, so uploaded bytes dominate wall time
    (plus a fixed first-transfer penalty while the terminal drains prior
    session teardown - a tiny warm-up put starts that clock immediately).
  * The device's SWDGE indirect-DMA gather cannot batch multiple offsets
    per partition (multi-index offset APs scramble on HW), so an on-device
    neighbor gather costs ~1 us of Pool time per 128 edges (~8.5 ms/core)
    plus minutes of walrus compile for the thousands of DMA instructions.

So: vertices are sharded across cores; the random neighbor gather is
resolved on the host during staging (np.take over packed per-vertex
tables, ~0.3 s for all 8M edges); each core receives contiguous
partition-major streams and the device runs a ~130-instruction streaming
kernel: HWDGE loads, fp16 DVE edge math (int8 R_j payload dequantized on
the fly), a strided reduce over K=8, fp16 rhs store. Compiles in ~2 s and
executes in ~0.2 s including download.

Per-edge payload is 17 B: [p_j (3 fp16) | s_j (fp16)] packed stride-4 and
R_j as int8 scaled by the per-vertex s_j = max|R_j|/127. End-to-end
max-normalized error vs the f32 reference: ~1.8e-3 (tolerance 2e-2).

Wall-clock ordering inside kernel(): issue a warm-up transfer at t=0,
stage cores and fire async device_puts as each finishes, then bass-build
and AOT-compile (walrus) while the link drains, then execute on the
pre-uploaded shards. A conservative fallback reruns everything through
run_bass_kernel_spmd if the AOT fast path fails.
"""
import time

import numpy as np

import jax
from jax.sharding import Mesh, NamedSharding, PartitionSpec
from jax.experimental.shard_map import shard_map

from concourse import bass, bacc, bass2jax, mybir, tile

K = 8
NCORES = 8
D = 12           # fp16 local row: p(3) + R(9)
GRP = 123        # vertices per partition per group
NGRP = 8         # groups per core: 8*128*123 = 125952 >= 125000

LAST_EXEC_NS = None
LAST_RUN_WALL_S = None
LAST_STAGE_S = None
LAST_COMPILE_S = None
LAST_NEFF_S = None
LAST_UPLOAD_S = None
LAST_PATH = None

_IN_SHAPES = {
    "gps": ((128, NGRP * GRP * K * 4), np.float16),
    "gr8": ((128, NGRP * GRP * K * 9), np.int8),
    "wgt": ((128, NGRP * GRP * K), np.int8),
    "locps": ((128, NGRP * GRP * 4), np.float16),
    "locr8": ((128, NGRP * GRP * 9), np.int8),
}
W_SCALE = 0.5 / 127.0   # dequant for int8 weights, 0.5 rhs factor folded in


def build_kernel(ngrp, grp, num_devices):
    nc = bacc.Bacc("TRN2", target_bir_lowering=False, debug=False,
                   num_devices=num_devices)
    f16 = mybir.dt.float16
    i8 = mybir.dt.int8
    ek = grp * K
    shp = 128 * grp * ngrp
    gps = nc.dram_tensor("gps", [128, ngrp * ek * 4], f16,
                         kind="ExternalInput").ap()
    gr8 = nc.dram_tensor("gr8", [128, ngrp * ek * 9], i8,
                         kind="ExternalInput").ap()
    wgt = nc.dram_tensor("wgt", [128, ngrp * ek], i8,
                         kind="ExternalInput").ap()
    locps = nc.dram_tensor("locps", [128, ngrp * grp * 4], f16,
                           kind="ExternalInput").ap()
    locr8 = nc.dram_tensor("locr8", [128, ngrp * grp * 9], i8,
                           kind="ExternalInput").ap()
    rhs = nc.dram_tensor("rhs", [shp, 3], f16, kind="ExternalOutput").ap()

    with tile.TileContext(nc) as tc, tc.tile_pool(name="sbuf", bufs=2) as pool:
        for g in range(ngrp):
            ps_t = pool.tile([128, ek, 4], f16, tag="gps")
            r8_t = pool.tile([128, ek, 9], i8, tag="gr8")
            w_t = pool.tile([128, ek], i8, tag="wgt")
            lps_t = pool.tile([128, grp, 4], f16, tag="locps")
            lr8_t = pool.tile([128, grp, 9], i8, tag="locr8")
            ri_t = pool.tile([128, grp, 9], f16, tag="ri")
            rs_t = pool.tile([128, ek, 9], f16, tag="rsc")
            df_t = pool.tile([128, ek, 3], f16, tag="diff")
            s_t = pool.tile([128, ek, 9], f16, tag="ssum")
            u_t = pool.tile([128, ek, 3], f16, tag="utmp")
            t_t = pool.tile([128, ek, 3], f16, tag="tacc")
            m_t = pool.tile([128, ek, 3], f16, tag="mout")
            o2_t = pool.tile([128, grp, 3], f16, tag="out")

            nc.sync.dma_start(out=ps_t[:],
                              in_=gps[:, g * ek * 4:(g + 1) * ek * 4])
            nc.sync.dma_start(out=r8_t[:],
                              in_=gr8[:, g * ek * 9:(g + 1) * ek * 9])
            nc.sync.dma_start(out=w_t[:], in_=wgt[:, g * ek:(g + 1) * ek])
            nc.sync.dma_start(out=lps_t[:],
                              in_=locps[:, g * grp * 4:(g + 1) * grp * 4])
            nc.sync.dma_start(out=lr8_t[:],
                              in_=locr8[:, g * grp * 9:(g + 1) * grp * 9])

            # gathered p_j [128, ek, 3] (stride-4 rows of ps_t)
            gp = bass.AP(ps_t.tensor, ps_t[:].offset,
                         [ps_t[:].ap[0], (4, ek), (1, 3)])
            # per-edge scale s_j broadcast over the 9 R components
            sv = bass.AP(ps_t.tensor, ps_t[:].offset + 3,
                         [ps_t[:].ap[0], (4, ek), (0, 9)])
            # local p_i / s_i slices broadcast over k
            lp = bass.AP(lps_t.tensor, lps_t[:].offset,
                         [lps_t[:].ap[0], (4, grp), (0, K), (1, 3)])
            ls = bass.AP(lps_t.tensor, lps_t[:].offset + 3,
                         [lps_t[:].ap[0], (4, grp), (0, 9)])
            riv = bass.AP(ri_t.tensor, ri_t[:].offset,
                          [ri_t[:].ap[0], (9, grp), (0, K), (1, 9)])

            # R_i = int8 * s_i ;  R_j = int8 * s_j  (DVE converts int8)
            nc.vector.tensor_tensor(out=ri_t[:], in0=lr8_t[:], in1=ls,
                                    op=mybir.AluOpType.mult)
            nc.vector.tensor_tensor(out=rs_t[:], in0=r8_t[:], in1=sv,
                                    op=mybir.AluOpType.mult)
            # diff = p_i - p_j ;  S = R_i + R_j
            nc.vector.tensor_tensor(out=df_t[:], in0=lp,
                                    in1=gp, op=mybir.AluOpType.subtract)
            nc.vector.tensor_tensor(out=s_t[:], in0=riv,
                                    in1=rs_t[:], op=mybir.AluOpType.add)

            def s_col(c):
                return bass.AP(s_t.tensor, s_t[:].offset + c,
                               [s_t[:].ap[0], (9, ek), (3, 3)])

            def d_col(c):
                return bass.AP(df_t.tensor, df_t[:].offset + c,
                               [df_t[:].ap[0], (3, ek), (0, 3)])

            # t = S @ diff (column-wise accumulation)
            nc.vector.tensor_tensor(out=t_t[:], in0=s_col(0), in1=d_col(0),
                                    op=mybir.AluOpType.mult)
            nc.vector.tensor_tensor(out=u_t[:], in0=s_col(1), in1=d_col(1),
                                    op=mybir.AluOpType.mult)
            nc.vector.tensor_tensor(out=t_t[:], in0=t_t[:], in1=u_t[:],
                                    op=mybir.AluOpType.add)
            nc.vector.tensor_tensor(out=u_t[:], in0=s_col(2), in1=d_col(2),
                                    op=mybir.AluOpType.mult)
            nc.vector.tensor_tensor(out=t_t[:], in0=t_t[:], in1=u_t[:],
                                    op=mybir.AluOpType.add)

            # m = t * w8  (int8 weight, 0..127; broadcast over 3 comps)
            wv = bass.AP(w_t.tensor, w_t[:].offset,
                         [w_t[:].ap[0], (1, ek), (0, 3)])
            nc.vector.tensor_tensor(out=m_t[:], in0=t_t[:], in1=wv,
                                    op=mybir.AluOpType.mult)

            # reduce over k (innermost view axis); fp16 accumulation of 8
            # terms costs ~1e-3 of max |rhs| against the 2e-2 tolerance
            mv = bass.AP(m_t.tensor, m_t[:].offset,
                         [m_t[:].ap[0], (3 * K, grp), (1, 3), (3, K)])
            with nc.allow_low_precision(reason="k=8 fp16 sum, tol 2e-2"):
                nc.vector.tensor_reduce(out=o2_t[:], in_=mv,
                                        axis=mybir.AxisListType.X,
                                        op=mybir.AluOpType.add)
            # dequantize weights + the rhs 0.5 factor in one scalar multiply
            nc.vector.tensor_scalar_mul(o2_t[:], o2_t[:], W_SCALE)

            # contiguous store: partition p -> rhs rows [g*128*grp + p*grp ..)
            rhs_dst = bass.AP(rhs.tensor, g * 128 * grp * 3,
                              [(grp * 3, 128), (1, grp * 3)])
            nc.sync.dma_start(out=rhs_dst, in_=o2_t[:])
    nc.compile()
    return nc


def make_tables(xyz1, neighborList, weightMatrix, rotations, n):
    p16 = np.ascontiguousarray(xyz1[0]).astype(np.float16)
    r = np.ascontiguousarray(rotations).reshape(n, 9)
    s = (np.abs(r).max(axis=1) / 127.0).astype(np.float16)
    table_ps = np.concatenate([p16, s[:, None]], axis=1)  # [n, 4] fp16
    table_r8 = np.clip(np.round(r / s.astype(np.float32)[:, None]),
                       -127, 127).astype(np.int8)         # [n, 9]
    nbr = np.ascontiguousarray(neighborList).reshape(n, K).astype(np.int32)
    w8 = np.clip(np.round(np.ascontiguousarray(weightMatrix)
                          .reshape(n, K).astype(np.float32) * 127.0),
                 0, 127).astype(np.int8)
    return table_ps, table_r8, nbr, w8


def stage_core(tables, i0, i1):
    table_ps, table_r8, nbr, w8 = tables
    shp = 128 * GRP * NGRP
    base = np.arange(shp)
    sh = i1 - i0
    vid = base % sh + i0                                  # padded ids (wrap)
    pad_mask = base >= sh

    def perm(a2d):
        # [shp, W] in vertex order -> [128, NGRP, GRP, W] partition-major
        W = a2d.shape[1]
        return np.ascontiguousarray(
            a2d.reshape(NGRP, 128, GRP, W).transpose(1, 0, 2, 3)
            .reshape(128, NGRP * GRP * W))

    nb_flat = perm(nbr[vid]).ravel()
    w_c = w8[vid]
    w_c[pad_mask] = 0
    vid_flat = perm(vid[:, None]).ravel()
    return {
        "gps": np.take(table_ps, nb_flat, axis=0).reshape(128, -1),
        "gr8": np.take(table_r8, nb_flat, axis=0).reshape(128, -1),
        "wgt": perm(w_c),
        "locps": np.take(table_ps, vid_flat, axis=0).reshape(128, -1),
        "locr8": np.take(table_r8, vid_flat, axis=0).reshape(128, -1),
    }


def _exec_setup(nc):
    """Mirror run_bass_via_pjrt's multi-core path, AOT + presharded."""
    bass2jax.install_neuronx_cc_hook()
    partition_name = (nc.partition_id_tensor.name
                      if nc.partition_id_tensor else None)
    assert nc.dbg_addr is None
    in_names, out_names, out_avals = [], [], []
    for alloc in nc.m.functions[0].allocations:
        if not isinstance(alloc, mybir.MemoryLocationSet):
            continue
        name = alloc.memorylocations[0].name
        if alloc.kind == "ExternalInput":
            if name != partition_name:
                in_names.append(name)
        elif alloc.kind == "ExternalOutput":
            out_names.append(name)
            out_avals.append(jax.core.ShapedArray(
                tuple(alloc.tensor_shape), mybir.dt.np(alloc.dtype)))
    n_params = len(in_names)
    all_names = in_names + out_names
    if partition_name is not None:
        all_names = all_names + [partition_name]

    def _body(*args):
        operands = list(args)
        if partition_name is not None:
            operands.append(bass2jax.partition_id_tensor())
        outs = bass2jax._bass_exec_p.bind(
            *operands,
            out_avals=tuple(out_avals),
            in_names=tuple(all_names),
            out_names=tuple(out_names),
            lowering_input_output_aliases=(),
            sim_require_finite=True,
            sim_require_nnan=True,
            nc=nc,
        )
        return tuple(outs)

    devices = jax.devices()[:NCORES]
    mesh = Mesh(np.asarray(devices), ("core",))
    spec = PartitionSpec("core")
    n_out = len(out_names)
    sharded = jax.jit(
        shard_map(_body, mesh=mesh, in_specs=(spec,) * (n_params + n_out),
                  out_specs=(spec,) * n_out, check_rep=False),
        donate_argnums=tuple(range(n_params, n_params + n_out)),
        keep_unused=True,
    )
    return sharded, in_names, out_names, out_avals, mesh, spec, devices


_PROG = None       # (nc, setup) — bass program + jit wrapper, per process
_COMPILED = None   # AOT-compiled executable, per process


def _get_prog():
    global _PROG
    if _PROG is None:
        nc = build_kernel(NGRP, GRP, NCORES)
        _PROG = (nc, _exec_setup(nc))
    return _PROG


def _get_compiled(setup):
    """AOT-compile the sharded program once per process (cached)."""
    global _COMPILED
    if _COMPILED is not None:
        return _COMPILED
    sharded, in_names, out_names, out_avals, mesh, spec, devices = setup
    nds = NamedSharding(mesh, spec)
    global_avals = []
    for name in in_names:
        pc_shape, pc_dtype = _IN_SHAPES[name]
        global_avals.append(jax.ShapeDtypeStruct(
            (NCORES * pc_shape[0],) + pc_shape[1:], pc_dtype, sharding=nds))
    for av in out_avals:
        global_avals.append(jax.ShapeDtypeStruct(
            (NCORES * av.shape[0],) + av.shape[1:], av.dtype, sharding=nds))
    _COMPILED = sharded.lower(*global_avals).compile()
    return _COMPILED


def kernel(xyz1, xyz2, neighborList, numNeighbors, accnumNeighbors,
           weightMatrix, rotations, arapWeight, trace=False):
    global LAST_RUN_WALL_S, LAST_STAGE_S, LAST_COMPILE_S
    global LAST_NEFF_S, LAST_UPLOAD_S, LAST_PATH
    n = xyz1.shape[1]
    sh = n // NCORES
    shp = 128 * GRP * NGRP
    assert shp >= sh, (shp, sh)
    shard = [(c * sh, (c + 1) * sh) for c in range(NCORES)]

    # warm-up transfer: the first put of a process can stall for tens of
    # seconds while the terminal drains prior-session teardown; start that
    # clock before any CPU work.
    devices = jax.devices()[:NCORES]
    _warm = jax.device_put(np.zeros(1024, np.float32), devices[0])

    # stage each core and fire its uploads immediately (async): the link
    # drains while the next core stages and later while walrus compiles
    t0 = time.time()
    tables = make_tables(xyz1, neighborList, weightMatrix, rotations, n)
    core_maps = []
    shard_arrays = {name: [] for name in _IN_SHAPES}
    for c, (i0, i1) in enumerate(shard):
        cc = stage_core(tables, *shard[c])
        core_maps.append(cc)
        for name in shard_arrays:
            shard_arrays[name].append(jax.device_put(cc[name], devices[c]))
    t1 = time.time()
    LAST_STAGE_S = t1 - t0

    try:
        nc, setup = _get_prog()
        _, in_names, out_names, out_avals, mesh, spec, _devs = setup
        nds = NamedSharding(mesh, spec)
        t2 = time.time()
        LAST_COMPILE_S = t2 - t1
        out_zero_arrays = []
        for av in out_avals:
            z = np.zeros(av.shape, av.dtype)
            out_zero_arrays.append([jax.device_put(z, d) for d in devices])
        compiled = _get_compiled(setup)
        t3 = time.time()
        LAST_NEFF_S = t3 - t2

        for arrs in shard_arrays.values():
            for a in arrs:
                a.block_until_ready()
        for arrs in out_zero_arrays:
            for a in arrs:
                a.block_until_ready()
        t4 = time.time()
        LAST_UPLOAD_S = t4 - t3

        def _global(shards, pc_shape, dtype):
            gshape = (NCORES * pc_shape[0],) + tuple(pc_shape[1:])
            return jax.make_array_from_single_device_arrays(
                gshape, nds, shards)

        args = []
        for name in in_names:
            pc_shape, pc_dtype = _IN_SHAPES[name]
            args.append(_global(shard_arrays[name], pc_shape, pc_dtype))
        for i, av in enumerate(out_avals):
            args.append(_global(out_zero_arrays[i], av.shape, av.dtype))

        out_arrs = compiled(*args)
        rhs = np.asarray(out_arrs[0])
        LAST_RUN_WALL_S = time.time() - t4
        LAST_PATH = "aot"
    except Exception:
        # conservative fallback: stock SPMD runner (re-uploads everything)
        from concourse.bass_utils import run_bass_kernel_spmd
        nc = build_kernel(NGRP, GRP, NCORES)
        t3 = time.time()
        res = run_bass_kernel_spmd(nc, core_maps, list(range(NCORES)),
                                   trace=trace)
        LAST_RUN_WALL_S = time.time() - t3
        LAST_PATH = "fallback"
        rhs = np.concatenate([res.results[c]["rhs"] for c in range(NCORES)],
                             axis=0)

    rhs_g = np.asarray(rhs).reshape(NCORES, shp, 3)
    parts = [rhs_g[c, :sh] for c in range(NCORES)]
    return np.concatenate(parts, axis=0).astype(np.float32)
